# revision 30
# baseline (speedup 1.0000x reference)
"""PoolHiddenNet-style GNN message passing kernel for 8 Trainium2 cores.

Math (per group s of S=32, P=64 peds, uniform groups):
  rel[i,j]  = obs[j] - obs[i]                         (P^2, 16)
  emb       = rel @ W_sp + b_sp                       (P^2, 512)
  x_a       = tw * emb          tw[n, t*64+k] = twq[n, t*2+k%2]
  x1        = relu(bn([x_a, h1] @ W1 + b1))           (P^2, 512)
  x2        = relu(bn(x1 @ W2 + b2))                  (P^2, 1024)
  out       = max over j                              (P, 1024)

Key transforms used here:
  * b1/b2 cancel inside train-mode BN (bias shifts the mean equally).
  * tw*emb @ W1a == z @ C with z[n, q*16+r] = twq[n,q]*rel[n,r] and
    C[q*16+r, d] = sum_{f: q(f)=q} W_sp[r,f] W1a[f,d]  (K 576 -> 320).
    b_sp contributes twq @ Cb with Cb[q,d] = sum_{f:q(f)=q} b_sp[f] W1a[f,d].
  * h1 @ W1b uses h1T = hT broadcast over i (only 64 distinct rows);
    the broadcast runs on DVE from a resident SBUF copy of h (a DMA
    expansion would need 4k tiny descriptors per group).
  * BN1 apply+relu is folded away: with a=g1/std1>0, t=-(be1-m*a)/a,
    relu(a*x+b) = a*max(x,t)+b. The kernel stores y=max(x1pre,t) (one
    4x-mode DVE op with accum giving colsum(y) for mean2), scales W2's
    rows by a instead (W2a = diag(a) @ W2), and drops the +b entirely:
    BN2 centers x2 per feature, so a constant row offset is invisible
    (the final relu happens after BN2, which uses the consistent
    mean2 = colsum(y) @ W2a / N).
  * BN2 apply is monotone (gamma*rsqrt > 0), so max-pool first, then
    apply BN+relu on the pooled (P, 1024) values only.
  * x2 is evicted once per PSUM tile by ACT (Copy activation whose
    accum_out gives colsum(x2) -> mean2 directly, in the right layout:
    no thin matmul, no DRAM bounce). From the bf16 SBUF copy, DVE runs
    a 2x-mode pairwise tensor_max cascade for the j-max (cheaper than
    1x tensor_reduce) and sum-of-squares passes split ~40/60 between
    ACT (Square+accum) and DVE (scalar_tensor_tensor+accum) to balance
    the two evacuation engines. (tensor_tensor_reduce faults on HW;
    stt is the working accum path. GpSimd stock tensor ops measure
    ~15x slower than DVE AND stall DVE via the shared SBUF port -
    keep bulk elementwise off GpSimd.)
  * Everything runs feature-on-partition (transposed activations); the
    output leaves the device feature-major ([D2, G*P]) and the host
    transposes during the unshard, so no PE transposes sit behind the
    BN2 chain at group boundaries.

Sharding: data-parallel over S; core c handles groups 4c..4c+3.
"""

import os
import numpy as np
import ml_dtypes

S, P = 32, 64
PP = P * P                  # 4096
OBS, EMB, HDIM = 8, 64, 64
D1, D2 = 512, 1024
NCORES = 8
G = S // NCORES             # 4 groups per core
EPS = 1e-5

BF16 = ml_dtypes.bfloat16
# matmul/operand dtype for the main chain ("bf16" or "f32")
MM_DTYPE = os.environ.get("KERNEL_MM_DTYPE", "bf16")

_PROG_CACHE = {}
LAST_RESULTS = None


def _np_mm_dtype():
    return np.float32 if MM_DTYPE == "f32" else BF16


def build_program():
    """Build (and compile) the per-core Bass program. Returns nc."""
    import concourse.bacc as bacc
    import concourse.mybir as mybir
    import concourse.tile as tile
    from concourse import masks

    f32 = mybir.dt.float32
    f16 = mybir.dt.float16
    DT = mybir.dt.float32 if MM_DTYPE == "f32" else mybir.dt.bfloat16
    AF = mybir.ActivationFunctionType
    ALU = mybir.AluOpType

    nc = bacc.Bacc("TRN2", target_bir_lowering=False, debug=False)

    # ---- DRAM I/O ----
    d_obsT = nc.dram_tensor("obsT", [G * 16, P], f32, kind="ExternalInput")
    d_twqT = nc.dram_tensor("twqT", [16, G, PP], DT, kind="ExternalInput")
    d_hT = nc.dram_tensor("hT", [HDIM, G, P], DT, kind="ExternalInput")
    d_C = nc.dram_tensor("Csb", [128, 2, D1], DT, kind="ExternalInput")
    d_CbW = nc.dram_tensor("CbW", [HDIM + 16, D1], DT, kind="ExternalInput")
    d_W2 = nc.dram_tensor("W2sb", [128, 4, D2], DT, kind="ExternalInput")
    d_g1 = nc.dram_tensor("g1c", [128, 4], f32, kind="ExternalInput")
    d_nbg1 = nc.dram_tensor("nbg1c", [128, 4], f32, kind="ExternalInput")
    d_g2 = nc.dram_tensor("g2c", [128, 8], f32, kind="ExternalInput")
    d_be2 = nc.dram_tensor("be2c", [128, 8], f32, kind="ExternalInput")
    d_out = nc.dram_tensor("out", [D2, G * P], f32, kind="ExternalOutput")

    HF = PP // 2

    with tile.TileContext(nc) as tc:
        with (
            tc.tile_pool(name="singles", bufs=1) as singles,
            tc.tile_pool(name="work", bufs=2) as work,
            tc.tile_pool(name="stat", bufs=2) as stat,
            tc.tile_pool(name="stat1", bufs=1) as stat1,
            tc.tile_pool(name="xps", bufs=4) as xps,
            tc.tile_pool(name="stat4", bufs=4) as stat4,
            tc.tile_pool(name="psmm", bufs=4, space="PSUM") as psmm,
            tc.tile_pool(name="dscr", bufs=2, space="DRAM") as dscr,
        ):
            # ---- early: the z-operand critical path ----
            # obsT2[ih*64+r, j] = obs feature r of ped j (both i-halves)
            obsT2 = singles.tile([128, P], f32)
            obsI = singles.tile([128, P // 2], f32)
            for a in range(2):
                nc.sync.dma_start(
                    out=obsT2[a * 64:(a + 1) * 64, :], in_=d_obsT.ap())
                # obsI[ih*64+r, il] = obs feature r of ped i=ih*32+il
                nc.sync.dma_start(
                    out=obsI[a * 64:(a + 1) * 64, :],
                    in_=d_obsT.ap()[:, a * 32:(a + 1) * 32])

            # rel2[(ih,r), (il,j)] = obs[r, j] - obs[r, ih*32+il]; bounced
            # to DRAM so the z DMA B-expansion can partition-broadcast.
            rel2 = work.tile([128, P * (P // 2)], DT, tag="asb")
            nc.vector.tensor_sub(
                rel2[:].rearrange("p (i j) -> p i j", j=P),
                obsT2[:, None, :].broadcast_to((128, P // 2, P)),
                obsI[:, :, None].broadcast_to((128, P // 2, P)),
            )
            reld = dscr.tile([G * 16, PP], DT, tag="reld")
            for a in range(2):
                nc.sync.dma_start(
                    out=reld[:, a * HF:(a + 1) * HF],
                    in_=rel2[a * 64:(a + 1) * 64, :])

            Csb = singles.tile([128, 2, D1], DT)
            CbW = singles.tile([HDIM + 16, D1], DT)
            hsb = singles.tile([HDIM, G, P], DT)
            for t_sb, t_dr in [(Csb, d_C), (CbW, d_CbW), (hsb, d_hT)]:
                nc.scalar.dma_start(out=t_sb[:], in_=t_dr.ap())

            def z_build(g, first=False):
                """k3 prefetch + z operand DMA-expansion + z multiply.
                first=True splits the h=0 expansion into quarters so the
                first matmuls can start ~5us earlier at kernel start."""
                # k3: third x1 K-chunk = [h1 (64 rows); twq (16 rows)];
                # h1T[hd, i*64+j] = hT[hd, g, j]: DVE broadcast from hsb
                # (h first - DVE writes need a 32-aligned start partition).
                k3 = work.tile([HDIM + 16, PP], DT, tag="k3")
                nc.sync.dma_start(out=k3[HDIM:, :], in_=d_twqT.ap()[:, g, :])
                nc.vector.tensor_copy(
                    k3[0:HDIM, :].rearrange("p (i j) -> p i j", j=P),
                    hsb[:, g, None, :].broadcast_to((HDIM, P, P)))

                # zT[q*16+r, n] = twqT[q, n] * relT[r, n]
                zT = work.tile([128, 2, PP], DT, tag="zT")
                nh_split = 2
                HW_ = PP // nh_split
                for h in range(nh_split):
                    nq = 4 if (first and h == 0) else 1
                    qw = HW_ // nq
                    B_h = work.tile([128, HW_], DT, tag="bsb")
                    for q in range(nq):
                        nc.sync.dma_start(
                            out=B_h[:, q * qw:(q + 1) * qw],
                            in_=reld[None, g * 16:g * 16 + 16,
                                     h * HW_ + q * qw:h * HW_ + (q + 1) * qw]
                            .broadcast_to((8, 16, qw)))
                    for kc in range(2):
                        A_h = work.tile([128, HW_], DT, tag="asb")
                        for q in range(nq):
                            nc.sync.dma_start(
                                out=A_h[:, q * qw:(q + 1) * qw],
                                in_=d_twqT.ap()[8 * kc:8 * kc + 8, g, None,
                                                h * HW_ + q * qw:
                                                h * HW_ + (q + 1) * qw]
                                .broadcast_to((8, 16, qw)))
                        nm = max(nq, 2)
                        mw = HW_ // nm
                        for q in range(nm):
                            nc.vector.tensor_mul(
                                zT[:, kc, h * HW_ + q * mw:h * HW_ + (q + 1) * mw],
                                A_h[:, q * mw:(q + 1) * mw],
                                B_h[:, q * mw:(q + 1) * mw])
                return zT, k3

            def x1_mms_dch(x1, zT, k3, dch):
                """One dch of x1pre = z@C + [twq; h1]@CbW: matmuls,
                raw bf16 evict (ACT), bn_stats from the evicted bf16
                (DVE). The stats chain + apply live in x1_post_dch,
                emitted later (interleaved into the previous group's
                x2 phase) so the tiny ACT sqrt never head-of-line
                blocks x2 evicts, and so PE alternates small x1 MM
                blocks with x2 MM blocks."""
                if True:
                    d0 = dch * 128
                    stats1 = stat4.tile([128, 8, 6], f32, tag="stats1")
                    for nc2 in range(4):
                        px = psmm.tile([128, 2, 512], f32, tag="mm")
                        # kc-outer so consecutive matmuls share the lhsT
                        for nh in range(2):
                            n0 = nc2 * 1024 + nh * 512
                            nc.tensor.matmul(px[:, nh, :],
                                             Csb[:, 0, d0:d0 + 128],
                                             zT[:, 0, n0:n0 + 512],
                                             start=True, stop=False)
                        for nh in range(2):
                            n0 = nc2 * 1024 + nh * 512
                            nc.tensor.matmul(px[:, nh, :],
                                             Csb[:, 1, d0:d0 + 128],
                                             zT[:, 1, n0:n0 + 512],
                                             start=False, stop=False)
                        for nh in range(2):
                            n0 = nc2 * 1024 + nh * 512
                            nc.tensor.matmul(px[:, nh, :],
                                             CbW[:, d0:d0 + 128],
                                             k3[:, n0:n0 + 512],
                                             start=False, stop=True)
                        nc.scalar.copy(
                            out=x1[:, dch, nc2 * 1024:(nc2 + 1) * 1024],
                            in_=px[:].rearrange("p a b -> p (a b)"))
                        for nh in range(2):
                            nc.vector.bn_stats(
                                out=stats1[:, nc2 * 2 + nh, :],
                                in_=x1[:, dch,
                                       nc2 * 1024 + nh * 512:
                                       nc2 * 1024 + (nh + 1) * 512])
                return stats1

            def x1_post_dch(x1, W2a, stats1, dch):
                """BN1 stats -> t1/gam1; W2a row-scale; in-place max."""
                if True:
                    mv1 = stat.tile([128, 2], f32, tag="mv1")
                    nc.vector.bn_aggr(out=mv1[:], in_=stats1[:])
                    std1 = stat.tile([128, 1], f32, tag="std1")
                    gam1 = stat.tile([128, 1], f32, tag="gam1")
                    t1 = stat.tile([128, 1], f32, tag="t1")
                    nc.scalar.activation(out=std1[:], in_=mv1[:, 1:2],
                                         func=AF.Sqrt, bias=eps_t[:])
                    # t1 = mean1 - (be1/g1)*std1  (nbg1c = -be1/g1)
                    nc.vector.scalar_tensor_tensor(
                        out=t1[:], in0=std1[:], scalar=nbg1c[:, dch:dch + 1],
                        in1=mv1[:, 0:1],
                        op0=ALU.mult, op1=ALU.add)
                    nc.vector.reciprocal(out=std1[:], in_=std1[:])
                    nc.vector.tensor_mul(gam1[:], g1c[:, dch:dch + 1], std1[:])
                    # W2a rows (K block = dch) scaled by gam1
                    nc.vector.tensor_scalar_mul(
                        W2a[:, dch], W2sb[:, dch], gam1[:])
                    # y = max(x1pre, t1) in place (4x-mode DVE)
                    nc.vector.tensor_scalar(
                        out=x1[:, dch, :], in0=x1[:, dch, :],
                        scalar1=t1[:], scalar2=None, op0=ALU.max)

            def x2_phase(g, x1, W2a, cb=None):
                # x2 = y @ W2a. ACT evicts each px to bf16 SBUF with
                # accum_out = colsum(x2) (-> mean2, in the right layout -
                # no thin matmul, no DRAM bounce). DVE then reads the
                # bf16 copy at 2x: tensor_tensor_reduce for sumsq and a
                # pairwise tensor_max cascade for the j-max.
                ssq2 = stat.tile([128, 8, 4], f32, tag="ssq2")
                s2n = stat.tile([128, 8, 4], f32, tag="s2n")
                pooled = stat.tile([128, 8, P], f32, tag="pooled")
                for dch in range(8):
                    if cb is not None:
                        cb(dch)
                    d0 = dch * 128
                    mc1 = stat1.tile([128, 4, 2, 8, 32], DT, tag="mc1")
                    for nc2 in range(4):
                        px = psmm.tile([128, 2, 512], f32, tag="mm")
                        # kc-outer so consecutive matmuls share the lhsT
                        for kc in range(4):
                            for nh in range(2):
                                n0 = nc2 * 1024 + nh * 512
                                nc.tensor.matmul(
                                    px[:, nh, :], W2a[:, kc, d0:d0 + 128],
                                    x1[:, kc, n0:n0 + 512],
                                    start=(kc == 0), stop=(kc == 3))
                        x2s = xps.tile([128, 1024], DT, tag="x2s")
                        nc.scalar.activation(
                            out=x2s[:], in_=px[:].rearrange("p a b -> p (a b)"),
                            func=AF.Copy,
                            accum_out=s2n[:, dch, nc2:nc2 + 1])
                        sqscr = stat1.tile([128, 1024], DT, tag="sqscr")
                        if ((dch * 4 + nc2) % 7 < 3 and dch < 5) or (dch >= 6 and nc2 % 2 == 0):
                            # ~1/3 of the sumsq passes on ACT (Square),
                            # the rest on DVE - balances the two engines
                            nc.scalar.activation(
                                out=sqscr[:], in_=x2s[:], func=AF.Square,
                                accum_out=ssq2[:, dch, nc2:nc2 + 1])
                        else:
                            nc.vector.scalar_tensor_tensor(
                                out=sqscr[:], in0=x2s[:], scalar=1.0,
                                in1=x2s[:], op0=ALU.mult, op1=ALU.mult,
                                accum_out=ssq2[:, dch, nc2:nc2 + 1])
                        x2sr = x2s[:].rearrange("p (a i j) -> p a i j",
                                                a=2, j=P)
                        nc.vector.tensor_max(
                            mc1[:, nc2], x2sr[:, :, :, 0:P // 2],
                            x2sr[:, :, :, P // 2:P])
                    # bf16 cascade: j 32 -> 1 over the whole dch
                    m32 = mc1[:].rearrange("p a b c j -> p (a b c) j")
                    mc2 = stat1.tile([128, 64, 16], DT, tag="mc2")
                    nc.vector.tensor_max(mc2[:], m32[:, :, 0:16], m32[:, :, 16:32])
                    mc3 = stat1.tile([128, 64, 8], DT, tag="mc3")
                    nc.vector.tensor_max(mc3[:], mc2[:, :, 0:8], mc2[:, :, 8:16])
                    mc4 = stat1.tile([128, 64, 4], DT, tag="mc4")
                    nc.vector.tensor_max(mc4[:], mc3[:, :, 0:4], mc3[:, :, 4:8])
                    mc5 = stat1.tile([128, 64, 2], DT, tag="mc5")
                    nc.vector.tensor_max(mc5[:], mc4[:, :, 0:2], mc4[:, :, 2:4])
                    nc.vector.tensor_max(
                        pooled[:, dch, :],
                        mc5[:, :, 0:1].rearrange("p f o -> p (f o)"),
                        mc5[:, :, 1:2].rearrange("p f o -> p (f o)"))
                return ssq2, s2n, pooled

            def x2_finish(g, ssq2, s2n, pooled):
                # mean2 = colsum(x2)/N; var2 = sumsq/N - mean2^2
                mean2 = stat.tile([128, 8], f32, tag="mean2")
                nc.vector.reduce_sum(mean2[:], s2n[:], axis=mybir.AxisListType.X)
                nc.vector.tensor_scalar_mul(mean2[:], mean2[:], 1.0 / PP)
                ssqt = stat.tile([128, 8], f32, tag="ssqt")
                nc.vector.reduce_sum(ssqt[:], ssq2[:], axis=mybir.AxisListType.X)
                m2sq = stat.tile([128, 8], f32, tag="m2sq")
                nc.vector.tensor_mul(m2sq[:], mean2[:], mean2[:])
                var2 = stat.tile([128, 8], f32, tag="var2")
                nc.vector.scalar_tensor_tensor(
                    out=var2[:], in0=ssqt[:], scalar=1.0 / PP, in1=m2sq[:],
                    op0=mybir.AluOpType.mult, op1=mybir.AluOpType.subtract)
                std2 = stat.tile([128, 8], f32, tag="std2")
                gam2 = stat.tile([128, 8], f32, tag="gam2")
                bet2 = stat.tile([128, 8], f32, tag="bet2")
                nc.scalar.activation(out=std2[:], in_=var2[:],
                                     func=AF.Sqrt, bias=eps_t[:])
                nc.vector.reciprocal(out=std2[:], in_=std2[:])
                nc.vector.tensor_mul(gam2[:], g2c[:], std2[:])
                nc.vector.tensor_mul(bet2[:], mean2[:], gam2[:])
                nc.vector.tensor_sub(bet2[:], be2c[:], bet2[:])

                # BN2 apply + relu on the pooled values (one ACT op/dch);
                # output stays feature-major - d_out is [D2, G*P] and the
                # host transposes during the unshard (free for HW time).
                outT = stat1.tile([128, 8, P], f32, tag="outT")
                for dch in range(8):
                    nc.scalar.activation(
                        out=outT[:, dch], in_=pooled[:, dch], func=AF.Relu,
                        bias=bet2[:, dch:dch + 1], scale=gam2[:, dch:dch + 1])
                nc.sync.dma_start(
                    out=d_out.ap().rearrange("(c p) n -> p c n", c=8)
                    [:, :, g * P:(g + 1) * P],
                    in_=outT[:])

            n_groups = int(os.environ.get("KERNEL_GROUPS", G))
            zks = [z_build(0, first=True)]
            if n_groups > 1:
                zks.append(z_build(1))

            # remaining constants (off the startup critical path)
            W2sb = singles.tile([128, 4, D2], DT)
            g1c = singles.tile([128, 4], f32)
            nbg1c = singles.tile([128, 4], f32)
            g2c = singles.tile([128, 8], f32)
            be2c = singles.tile([128, 8], f32)
            eps_t = singles.tile([128, 1], f32)
            for t_sb, t_dr in [
                (W2sb, d_W2), (g1c, d_g1), (nbg1c, d_nbg1),
                (g2c, d_g2), (be2c, d_be2),
            ]:
                nc.scalar.dma_start(out=t_sb[:], in_=t_dr.ap())
            nc.vector.memset(eps_t[:], EPS)

            # group 0 prologue: x1 fully before its x2
            x1c = work.tile([128, 4, PP], DT, tag="x1")
            W2c = work.tile([128, 4, D2], DT, tag="W2a")
            st0 = [x1_mms_dch(x1c, *zks[0], dch) for dch in range(4)]
            for dch in range(4):
                x1_post_dch(x1c, W2c, st0[dch], dch)

            for g in range(n_groups):
                if g + 1 < n_groups:
                    # next group's x1 matmuls ahead of this group's x2;
                    # the BN1 stats chains + applies are emitted mid-x2
                    # (so the tiny ACT sqrts never head-of-line-block
                    # the x2 PSUM evicts)
                    zTn, k3n = zks[g + 1]
                    x1n = work.tile([128, 4, PP], DT, tag="x1")
                    W2n = work.tile([128, 4, D2], DT, tag="W2a")
                    stn = [x1_mms_dch(x1n, zTn, k3n, k) for k in range(4)]
                    if g + 2 < n_groups:
                        zks.append(z_build(g + 2))

                    def cb(step, x1n=x1n, W2n=W2n, stn=stn):
                        if step == 4:
                            for k in range(4):
                                x1_post_dch(x1n, W2n, stn[k], k)
                else:
                    cb = None
                ctx2 = x2_phase(g, x1c, W2c, cb=cb)
                x2_finish(g, *ctx2)
                if g + 1 < n_groups:
                    x1c, W2c = x1n, W2n

    nc.compile()
    return nc


def _host_prepare(inputs):
    """Slice/permute full inputs into 8 per-core in_maps (host-side)."""
    dtm = _np_mm_dtype()
    f32 = np.float32

    h_states = np.asarray(inputs["h_states"], f32)
    traj = np.asarray(inputs["traj"], f32)
    traj_weight = np.asarray(inputs["traj_weight"], f32)
    W_sp = np.asarray(inputs["W_sp"], f32)
    b_sp = np.asarray(inputs["b_sp"], f32)
    W1 = np.asarray(inputs["W1"], f32)
    g1 = np.asarray(inputs["g1"], f32)
    be1 = np.asarray(inputs["be1"], f32)
    W2 = np.asarray(inputs["W2"], f32)
    g2 = np.asarray(inputs["g2"], f32)
    be2 = np.asarray(inputs["be2"], f32)

    # The relu(a*x+b) = a*max(x, -b/a) + b fold needs a = g1/std1 > 0.
    # setup_inputs() always emits g1 = ones; fall back would need a min.
    assert np.all(g1 > 0), "kernel's BN1 relu fold requires g1 > 0"

    # obs: (S, P, 16) with feature index t*2+c
    obs = np.transpose(traj[:OBS], (1, 0, 2)).reshape(S, P, OBS * 2)
    h = h_states.reshape(S, P, HDIM)

    # C fold: q(f) = (f//64)*2 + f%2
    f_idx = np.arange(EMB * OBS)
    qof = (f_idx // EMB) * 2 + (f_idx % 2)
    W1a, W1b = W1[:D1], W1[D1:]
    C = np.zeros((256, D1), f32)
    Cb = np.zeros((16, D1), f32)
    for q in range(16):
        m = qof == q
        C[q * 16:(q + 1) * 16] = W_sp[:, m] @ W1a[m]
        Cb[q] = b_sp[m] @ W1a[m]
    Csb = np.ascontiguousarray(C.reshape(2, 128, D1).transpose(1, 0, 2))
    W2sb = np.ascontiguousarray(W2.reshape(4, 128, D2).transpose(1, 0, 2))

    shared = {
        "Csb": Csb.astype(dtm),
        "CbW": np.concatenate([W1b, Cb], axis=0).astype(dtm),
        "W2sb": W2sb.astype(dtm),
        "g1c": np.ascontiguousarray(g1.reshape(4, 128).T),
        "nbg1c": np.ascontiguousarray(-(be1 / g1).reshape(4, 128).T),
        "g2c": np.ascontiguousarray(g2.reshape(8, 128).T),
        "be2c": np.ascontiguousarray(be2.reshape(8, 128).T),
    }

    in_maps = []
    for c in range(NCORES):
        sl = slice(c * G, (c + 1) * G)
        obsT = np.ascontiguousarray(
            obs[sl].transpose(0, 2, 1).reshape(G * 16, P))    # (G*16, P)
        twqT = np.ascontiguousarray(
            traj_weight[sl].transpose(3, 2, 0, 1).reshape(16, G, PP))
        hT = np.ascontiguousarray(h[sl].transpose(2, 0, 1))           # (64,G,P)
        in_maps.append({
            "obsT": obsT,
            "twqT": twqT.astype(dtm),
            "hT": hT.astype(dtm),
            **shared,
        })
    return in_maps


def kernel(**inputs) -> np.ndarray:
    global LAST_RESULTS
    from concourse import bass_utils

    if "prog" not in _PROG_CACHE:
        _PROG_CACHE["prog"] = build_program()
    nc = _PROG_CACHE["prog"]

    in_maps = _host_prepare(inputs)
    trace = bool(int(os.environ.get("KERNEL_TRACE", "0")))
    res = bass_utils.run_bass_kernel_spmd(
        nc, in_maps, core_ids=list(range(NCORES)), trace=trace)
    LAST_RESULTS = res
    out = np.concatenate(
        [np.asarray(res.results[c]["out"]).T for c in range(NCORES)], axis=0)
    return np.ascontiguousarray(out, dtype=np.float32)
